# revision 1
# baseline (speedup 1.0000x reference)
"""3-layer GCN (GCNConv x3) on Trainium (8 NeuronCores) via jax/XLA-neuron.

Sharding (per hint: partition nodes / replicate weights):
- Nodes partitioned into 8 contiguous shards; core c owns dst nodes
  [c*12500, (c+1)*12500) and computes exactly those output rows.
- Edges (incl. self-loops) bucketed by dst shard on host; each core gathers
  h[src] from a replicated per-layer activation table and scatter-adds into
  its own shard (jax.ops.segment_sum), chunked to 4096 edges per indirect op
  (neuronx-cc 16-bit semaphore limit on larger indirect loads).
- Propagation always runs in the 64-wide representation (A_hat commutes with
  the feature matmul):
    L1: t1 = x @ W1;   o1 = relu(A t1 + b1)
    L2: p2 = A o1;     o2 = relu(p2 @ W2 + b2)
    L3: t3 = o2 @ W3;  out = A t3 + b3
- Per (layer, core): ONE jitted module containing the whole chunked
  aggregation (minimizes dispatch round-trips); modules are shape-identical
  across cores, so only 3 neuron compilations happen (cached afterwards).
"""
import numpy as np

N = 100000
N_CORES = 8
SHARD = N // N_CORES
CH = 4096                       # edges per indirect op

_cache = {}


GROUP = 2


def _get(kind, G):
    """'group': acc += sum of G chunk segment-sums; finishers: relu/lin/l2."""
    key = (kind, G)
    if key in _cache:
        return _cache[key]
    import jax

    if kind == 'group':
        def grp(table, src, dst, nrm, acc):
            for k in range(G):
                msg = table[src[k]] * nrm[k][:, None]
                acc = acc + jax.ops.segment_sum(msg, dst[k], num_segments=SHARD)
            return acc
        fn = jax.jit(grp)
    elif kind == 'relu':
        fn = jax.jit(lambda acc, b: jax.nn.relu(acc + b))
    elif kind == 'lin':
        fn = jax.jit(lambda acc, b: acc + b)
    else:  # 'l2': t3 = relu((acc) W2 + b2) W3
        fn = jax.jit(lambda acc, W2, b2, W3:
                     jax.nn.relu(acc @ W2 + b2) @ W3)
    _cache[key] = fn
    return fn


def _agg(table_dev, edata_c, zero_dev, grp_fn, K):
    import jax
    acc = zero_dev
    s, d, n = edata_c
    for g in range(0, K, GROUP):
        acc = grp_fn(table_dev, s[g:g + GROUP], d[g:g + GROUP], n[g:g + GROUP], acc)
    return acc


def _dense():
    if 'dense' in _cache:
        return _cache['dense']
    import jax
    fn = jax.jit(lambda x, W: x @ W)
    _cache['dense'] = fn
    return fn


def _allgather(devs):
    if 'ag' in _cache:
        return _cache['ag']
    import jax
    import numpy as _np
    from jax.sharding import Mesh, PartitionSpec as P, NamedSharding
    from jax.experimental.shard_map import shard_map
    mesh = Mesh(_np.array(devs), ('core',))
    fn = jax.jit(shard_map(lambda s: jax.lax.all_gather(s, 'core', axis=0, tiled=True),
                           mesh=mesh, in_specs=P('core'), out_specs=P(None),
                           check_rep=False))
    sharding = NamedSharding(mesh, P('core'))
    _cache['ag'] = (fn, sharding)
    return _cache['ag']


def _gather_tables(shards_per_core, devs):
    """Device-side allgather of per-core [SHARD, F] device arrays.
    Returns per-core full [N, F] device arrays (no host round trip)."""
    import jax
    fn, sharding = _allgather(devs)
    F = shards_per_core[0].shape[1]
    glob = jax.make_array_from_single_device_arrays(
        (N, F), sharding, [s for s in shards_per_core])
    rep = fn(glob)
    by_dev = {sh.device: sh.data for sh in rep.addressable_shards}
    return [by_dev[devs[c]] for c in range(N_CORES)]


def kernel(x, edge_index, W1, b1, W2, b2, W3, b3):
    import jax

    x = np.asarray(x, np.float32)
    edge_index = np.asarray(edge_index)
    W1, b1, W2, b2, W3, b3 = (np.asarray(a, np.float32)
                              for a in (W1, b1, W2, b2, W3, b3))
    devs = jax.devices()[:N_CORES]

    # ---- host: degrees / norms (same normalized adjacency for all layers) ----
    src = edge_index[0].astype(np.int64)
    dst = edge_index[1].astype(np.int64)
    loop = np.arange(N, dtype=np.int64)
    src_f = np.concatenate([src, loop])
    dst_f = np.concatenate([dst, loop])
    deg = np.bincount(dst_f, minlength=N).astype(np.float32)
    dinv = np.where(deg > 0, 1.0 / np.sqrt(deg), 0.0).astype(np.float32)
    norm = (dinv[src_f] * dinv[dst_f]).astype(np.float32)

    # ---- host: shard edges by dst owner, pad to K*CH ----
    owner = dst_f // SHARD
    order = np.argsort(owner, kind='stable')
    src_s, dst_s, norm_s = src_f[order], dst_f[order], norm[order]
    counts = np.bincount(owner, minlength=N_CORES)
    offs = np.concatenate([[0], np.cumsum(counts)])
    K = int(np.ceil(counts.max() / (CH * 8))) * 8   # multiple of GROUP

    def put(c, arr):
        return jax.device_put(arr, devs[c])

    edata = []
    for c in range(N_CORES):
        a, b = offs[c], offs[c + 1]
        pad = K * CH - (b - a)
        s = np.concatenate([src_s[a:b], np.zeros(pad, np.int64)]).astype(np.int32)
        d = np.concatenate([dst_s[a:b] - c * SHARD, np.zeros(pad, np.int64)]).astype(np.int32)
        nr = np.concatenate([norm_s[a:b], np.zeros(pad, np.float32)]).astype(np.float32)
        edata.append((put(c, s.reshape(K, CH)), put(c, d.reshape(K, CH)),
                      put(c, nr.reshape(K, CH))))

    W2d = [put(c, W2) for c in range(N_CORES)]
    W3d = [put(c, W3) for c in range(N_CORES)]
    b1d = [put(c, b1) for c in range(N_CORES)]
    b2d = [put(c, b2) for c in range(N_CORES)]
    b3d = [put(c, b3) for c in range(N_CORES)]
    z64 = [put(c, np.zeros(64, np.float32)) for c in range(N_CORES)]

    grp = _get('group', GROUP)
    fin_relu = _get('relu', 0)
    fin_l2 = _get('l2', 0)
    fin_lin = _get('lin', 0)
    dense = _dense()
    zacc = [put(c, np.zeros((SHARD, 64), np.float32)) for c in range(N_CORES)]

    # L1 dense sharded + device allgather
    t1_sh = [dense(put(c, x[c * SHARD:(c + 1) * SHARD]), put(c, W1))
             for c in range(N_CORES)]
    t1_tab = _gather_tables(t1_sh, devs)

    # L1 aggregation
    acc = [_agg(t1_tab[c], edata[c], zacc[c], grp, K) for c in range(N_CORES)]
    o1_sh = [fin_relu(acc[c], b1d[c]) for c in range(N_CORES)]
    o1_tab = _gather_tables(o1_sh, devs)

    # L2: p2 = A o1, then relu(p2 W2 + b2) W3 fused on device
    acc = [_agg(o1_tab[c], edata[c], zacc[c], grp, K) for c in range(N_CORES)]
    t3_sh = [fin_l2(acc[c], W2d[c], b2d[c], W3d[c]) for c in range(N_CORES)]
    t3_tab = _gather_tables(t3_sh, devs)

    # L3 aggregation + b3
    acc = [_agg(t3_tab[c], edata[c], zacc[c], grp, K) for c in range(N_CORES)]
    out_sh = [fin_lin(acc[c], b3d[c]) for c in range(N_CORES)]
    out = np.concatenate([np.asarray(t) for t in out_sh], axis=0)
    return out.astype(np.float32)

